# revision 35
# baseline (speedup 1.0000x reference)
"""Trainium2 Bass kernel for fused GQA attention block (B=2, L=2048, D=2048,
H=16 q-heads, KV=4 kv-heads, HD=64, causal, QK-RMSNorm + RoPE).

Sharding (8 cores): core c -> batch b = c // 4, head-group g = c % 4
(query heads 4g..4g+3, kv head g). Each core computes its 4 heads'
attention and a partial output projection (256 of 1024 e-channels);
host sums the 4 partials per batch (outside the timed device program).

v2 design (vs baseline):
 - bf16 operands everywhere off-PSUM (x, weights, Q^T/K^T/V, p, aot, y);
   fp32 PSUM accumulation. Halves DMA and SBUF traffic; same PE rate.
 - pair transposes: one [128,128] is_transpose matmul per head-pair (and a
   duplicated-k one) directly yields the pair-stacked layout; single DVE
   copy into an interleaved persistent Q^T/K^T buffer.
 - causal masking via a single [128,128] triangular constant accumulated
   on the PE into the boundary sub-tile of diagonal score blocks; exp and
   the score/AV matmuls are trimmed to the valid q-window. No gpsimd
   affine_select, no DVE mask adds.
 - rstd = exp(-0.5*ln(ms+eps)): keeps every activation (Square, Ln, Exp,
   Copy) in one act table set -> no table reloads when phases interleave.
 - phases are software-pipelined at q-chunk granularity: attention of
   chunk qc is interleaved with QKV+rope of chunk qc+1 and the output
   projection of chunk qc-1, so the PE never sits behind softmax exps.
"""

import os

import numpy as np
from ml_dtypes import bfloat16

import concourse.bass as bass
import concourse.mybir as mybir
import concourse.tile as tile
from concourse import bacc
from concourse import bass_utils
from concourse.hw_specs import get_activation_tables
from concourse.masks import make_identity

F32 = mybir.dt.float32
BF16 = mybir.dt.bfloat16
AF = mybir.ActivationFunctionType
ALU = mybir.AluOpType

B, L, D = 2, 2048, 2048
H, KV, HD = 16, 4, 64
EPS = 1e-6
ROPE_BASE = 10000.0
N_CORES = 8
GQ = H // KV          # 4 query heads per core
LT = L // 128         # 16 l-tiles
DT = D // 128         # 16 d-tiles (contraction tiles for qkv proj)
TQ = 512              # q-chunk width for attention
NQC = L // TQ         # 4 q-chunks
NKB = L // 128        # 16 k-blocks
G5 = GQ + 1           # norm groups (4 q heads + 1 k head)
EW = (GQ + 2) * HD    # 384 qkv channels per core
EO = GQ * HD          # 256 output channels per core
NEG = -1e9

KOPT_ZENG = os.environ.get("KOPT_ZENG", "vava")  # staging engine per dc
# (gpsimd cannot read PSUM, so only v=DVE / a=Act are valid here)
KOPT_WEAVE = os.environ.get("KOPT_WEAVE", "1") == "1"


def _build_program(repeat=1):
    nc = bacc.Bacc("TRN2", target_bir_lowering=False, debug=False,
                   enable_asserts=False, num_devices=N_CORES)

    # DRAM I/O (per core). Host pre-tiles everything into DMA-friendly
    # fully-contiguous bf16 layouts.
    xt = nc.dram_tensor("xt", [LT, 128, D], BF16, kind="ExternalInput").ap()
    wqkT = nc.dram_tensor("wqkT", [D, EW], BF16, kind="ExternalInput").ap()
    woT = nc.dram_tensor("woT", [EO, D], BF16, kind="ExternalInput").ap()
    cos2 = nc.dram_tensor("cos2", [128, LT * 32], F32, kind="ExternalInput").ap()
    sin2 = nc.dram_tensor("sin2", [128, LT * 32], F32, kind="ExternalInput").ap()
    qw = nc.dram_tensor("qw", [128, GQ * HD], F32, kind="ExternalInput").ap()
    kw = nc.dram_tensor("kw", [128, HD], F32, kind="ExternalInput").ap()
    tri = nc.dram_tensor("tri", [128, 128], BF16, kind="ExternalInput").ap()
    y = nc.dram_tensor("y", [L, D], BF16, kind="ExternalOutput").ap()

    with tile.TileContext(nc) as tc:
        with (
            tc.tile_pool(name="consts", bufs=1) as consts,
            tc.tile_pool(name="wpool", bufs=1) as wpool,
            tc.tile_pool(name="xcolp", bufs=4) as xcolp,
            tc.tile_pool(name="work", bufs=4) as work,
            tc.tile_pool(name="persist", bufs=1) as persist,
            tc.tile_pool(name="pp", bufs=6) as pp,
            tc.tile_pool(name="zp", bufs=3) as zp,
            tc.tile_pool(name="ps_mm", bufs=2, space="PSUM") as ps_mm,
            tc.tile_pool(name="ps_sc", bufs=2, space="PSUM") as ps_sc,
            tc.tile_pool(name="ps_av", bufs=1, space="PSUM") as ps_av,
        ):
            # Pin the one act table that serves Square+Ln+Exp+Copy; the
            # auto-inserter would otherwise thrash between per-func tables
            # (1283ns per reload) in the interleaved schedule.
            tabs = list(get_activation_tables(nc.m.arch).items())
            set_id = next(i for i, (_n, fns) in enumerate(tabs)
                          if {AF.Ln, AF.Exp, AF.Square, AF.Copy} <= fns)
            nc.scalar.add_instruction(mybir.InstLoadActFuncSet(
                name=f"I-{nc.next_id()}", engine=mybir.EngineType.Activation,
                act_func_set_id=set_id))

            # ---- constants ----
            ident_b = consts.tile([128, 128], BF16, tag="identb")
            make_identity(nc, ident_b[:])
            tri_sb = consts.tile([128, 128], BF16, tag="tri")
            nc.sync.dma_start(tri_sb[:], tri[:])
            cos_sb = consts.tile([128, LT * 32], F32, tag="cos")
            sin_sb = consts.tile([128, LT * 32], F32, tag="sin")
            nc.sync.dma_start(cos_sb[:], cos2[:])
            nc.sync.dma_start(sin_sb[:], sin2[:])
            eps_sb = consts.tile([128, 1], F32, tag="eps")
            nc.vector.memset(eps_sb[:], EPS)
            # w5 = [qw x4 | kw] (qw already has HD^-0.5 folded in on host)
            w5_sb = consts.tile([128, G5 * HD], F32, tag="w5")
            nc.sync.dma_start(w5_sb[:, 0:GQ * HD], qw[:])
            nc.sync.dma_start(w5_sb[:, GQ * HD:G5 * HD], kw[:])
            # bf16 copies of the rope/norm constants: all-2-byte operands
            # unlock the DVE 2x perf mode on the rope chain
            w5_b = consts.tile([128, G5 * HD], BF16, tag="w5b")
            nc.vector.tensor_copy(w5_b[:], w5_sb[:])
            cos_b = consts.tile([128, LT * 32], BF16, tag="cosb")
            sin_b = consts.tile([128, LT * 32], BF16, tag="sinb")
            nc.vector.tensor_copy(cos_b[:], cos_sb[:])
            nc.vector.tensor_copy(sin_b[:], sin_sb[:])

            # ---- weights (bf16) ----
            wqk_sb = []
            for dt_i in range(DT):
                w = wpool.tile([128, EW], BF16, tag=f"wqk{dt_i}")
                nc.sync.dma_start(w[:], wqkT[dt_i * 128:(dt_i + 1) * 128, :])
                wqk_sb.append(w)
            wo_sb = []
            for et in range(2):
                w = wpool.tile([128, D], BF16, tag=f"wo{et}")
                nc.sync.dma_start(w[:], woT[et * 128:(et + 1) * 128, :])
                wo_sb.append(w)

            # ---- persistent attention operands ----
            # qk_all[p, lt, g, j]: g=0 -> pair0 (heads 0,1 chan-stacked),
            # g=1 -> pair1 (heads 2,3), g=2 -> k^T duplicated on both halves.
            qk_all = persist.tile([128, LT * 3 * 128], BF16, tag="qkall")
            qk_v = qk_all[:].rearrange("p (t g j) -> p t g j", g=3, j=128)
            # vt[p=k-pos, (lt, hd|ones)]
            vt_sb = persist.tile([128, LT * 128], BF16, tag="vt")
            ones_sb = consts.tile([128, HD], BF16, tag="ones")
            nc.vector.memset(ones_sb[:], 1.0)
            for i in range(LT):
                nc.vector.tensor_copy(
                    vt_sb[:, i * 128 + HD:(i + 1) * 128], ones_sb[:])
            aot_sb = [persist.tile([128, L], BF16, tag=f"aot{et}",
                                   name=f"aot{et}")
                      for et in range(2)]

            state = {"prev": None}

            def emit_transposes():
                if state["prev"] is None:
                    return
                lt, rq = state["prev"]
                state["prev"] = None
                tp = ps_mm.tile([128, 1024], BF16, tag="mm", name="tp")
                for g in range(3):
                    nc.tensor.matmul(
                        tp[:, g * 128:(g + 1) * 128],
                        rq[:, g * 128:(g + 1) * 128],
                        ident_b[:], is_transpose=True,
                        skip_group_check=True)
                nc.vector.tensor_copy(
                    qk_v[:, lt, :, :],
                    tp[:, 0:384].rearrange("p (g j) -> p g j", j=128))

            # ================= Phase 1: QKV + RMSNorm + RoPE =================
            def emit_p1(lt):
                xcol = xcolp.tile([128, D], BF16, tag="xcol")
                nc.sync.dma_start(xcol[:], xt[lt, :, :])
                qkv_ps = ps_mm.tile([128, 512], F32, tag="mm", name="qkv_ps")
                for dt_i in range(DT):
                    nc.tensor.matmul(
                        qkv_ps[:, 0:EW],
                        xcol[:, dt_i * 128:(dt_i + 1) * 128],
                        wqk_sb[dt_i][:],
                        start=(dt_i == 0), stop=(dt_i == DT - 1))
                emit_transposes()

                # stage q,k out of PSUM; V straight to persistent (bf16)
                q5 = work.tile([128, G5 * HD], F32, tag="q5")
                nc.vector.tensor_copy(q5[:], qkv_ps[:, 0:G5 * HD])
                nc.vector.tensor_copy(
                    vt_sb[:, lt * 128:lt * 128 + HD],
                    qkv_ps[:, G5 * HD:(G5 + 1) * HD])

                # RMS stats: one act Square + one DVE X-reduce
                sq = work.tile([128, G5 * HD], F32, tag="sq")
                nc.scalar.activation(sq[:], q5[:], AF.Square)
                ss = work.tile([128, 16], F32, tag="ss")
                nc.vector.tensor_reduce(
                    ss[:, 0:G5],
                    sq[:].rearrange("p (h e) -> p h e", e=HD),
                    axis=mybir.AxisListType.X, op=ALU.add)
                # rstd = exp(-0.5 * ln(ss/HD + eps)); Ln+Exp share the act
                # table with Square/Copy -> no table reloads anywhere.
                nc.scalar.activation(ss[:, 5:5 + G5], ss[:, 0:G5],
                                     AF.Ln, bias=eps_sb[:], scale=1.0 / HD)
                nc.scalar.activation(ss[:, 10:10 + G5], ss[:, 5:5 + G5],
                                     AF.Exp, scale=-0.5)

                # normalize * weight (5 groups at once); bf16 from here on
                # so the rope tensor_tensor chain runs in DVE 2x mode
                qn = work.tile([128, G5 * HD], BF16, tag="qn")
                nc.vector.tensor_tensor(
                    qn[:].rearrange("p (h e) -> p h e", e=HD),
                    q5[:].rearrange("p (h e) -> p h e", e=HD),
                    ss[:, 10:10 + G5, None].broadcast_to([128, G5, HD]),
                    op=ALU.mult)
                nc.vector.tensor_tensor(qn[:], qn[:], w5_b[:], op=ALU.mult)

                # RoPE on all 5 groups; outputs bf16 for 1cyc/row transposes
                cs = cos_b[:, lt * 32:(lt + 1) * 32]
                sn = sin_b[:, lt * 32:(lt + 1) * 32]
                csq = cs[:, None, :].broadcast_to([128, G5, 32])
                snq = sn[:, None, :].broadcast_to([128, G5, 32])
                rq = work.tile([128, 3 * 128], BF16, tag="rq")
                rqv = rq[:, 0:G5 * HD].rearrange("p (h e) -> p h e", e=HD)
                qnv = qn[:].rearrange("p (h e) -> p h e", e=HD)
                t1 = work.tile([128, G5 * 32], BF16, tag="t1")
                t1v = t1[:].rearrange("p (h e) -> p h e", e=32)
                t2 = work.tile([128, G5 * 32], BF16, tag="t2")
                t2v = t2[:].rearrange("p (h e) -> p h e", e=32)
                # low half: x1*cos - x2*sin
                nc.vector.tensor_tensor(t1v, qnv[:, :, 0:32], csq, op=ALU.mult)
                nc.vector.tensor_tensor(t2v, qnv[:, :, 32:64], snq, op=ALU.mult)
                nc.vector.tensor_tensor(rqv[:, :, 0:32], t1v, t2v,
                                        op=ALU.subtract)
                # high half: x1*sin + x2*cos
                nc.vector.tensor_tensor(t1v, qnv[:, :, 0:32], snq, op=ALU.mult)
                nc.vector.tensor_tensor(t2v, qnv[:, :, 32:64], csq, op=ALU.mult)
                nc.vector.tensor_tensor(rqv[:, :, 32:64], t1v, t2v, op=ALU.add)
                # duplicate roped k so its transpose fills both halves
                nc.vector.tensor_copy(rq[:, 320:384], rq[:, 256:320])

                state["prev"] = (lt, rq)

            # ================= Phase 2: attention =================
            # Spine is software-pipelined one block deep: the AV matmul for
            # block ci is emitted after the scores+exp of block ci+1, so the
            # PE streams the next block's scores while the act engine exps
            # the current one.
            av_state = {}
            pend_state = {}

            def p2_units(qc):
                units = []
                for pr in range(GQ // 2):
                    nkb = 4 * qc + 4  # causal: kb in [0, 4qc+3]
                    for ci in range(nkb):
                        units.append(("blk", pr, qc, ci, nkb))
                    units.append(("norm", pr, qc))
                return units

            def emit_av(pr, stop):
                ci, nkb, p_sb, kb, delta = pend_state.pop(pr)
                av_ps = av_state[pr]
                for sub in range(2):
                    nc.tensor.matmul(
                        av_ps[:, sub * TQ + delta:(sub + 1) * TQ],
                        vt_sb[:, kb * 128:(kb + 1) * 128],
                        p_sb[:, sub * TQ + delta:(sub + 1) * TQ],
                        start=(ci == 0), stop=stop)

            def emit_p2_unit(u):
                kind = u[0]
                if kind == "blk":
                    _, pr, qc, ci, nkb = u
                    kb = ci
                    delta = max(0, kb * 128 - qc * TQ)
                    diag = kb >= 4 * qc
                    if ci == 0:
                        av_state[pr] = ps_av.tile([128, 2 * TQ], F32, tag="av",
                                                  name=f"av{pr}")
                    sc_ps = ps_sc.tile([128, 2 * TQ], F32, tag="sc")
                    for sub in range(2):
                        qsl = qk_v[sub * 64:(sub + 1) * 64,
                                   4 * qc + delta // 128:4 * qc + 4, pr, :]
                        nc.tensor.matmul(
                            sc_ps[:, sub * TQ + delta:(sub + 1) * TQ],
                            qk_v[sub * 64:(sub + 1) * 64, kb, 2, :],
                            qsl,
                            start=True, stop=not diag)
                        if diag:
                            nc.tensor.matmul(
                                sc_ps[:, sub * TQ + delta:sub * TQ + delta + 128],
                                ident_b[:], tri_sb[:],
                                start=False, stop=True)
                    p_sb = pp.tile([128, 2 * TQ], BF16, tag="p")
                    sc3 = sc_ps[:].rearrange("p (s q) -> p s q", q=TQ)
                    p3 = p_sb[:].rearrange("p (s q) -> p s q", q=TQ)
                    nc.scalar.activation(p3[:, :, delta:TQ],
                                         sc3[:, :, delta:TQ], AF.Exp)
                    if pr in pend_state:
                        emit_av(pr, stop=False)
                    pend_state[pr] = (ci, nkb, p_sb, kb, delta)
                else:
                    _, pr, qc = u
                    emit_av(pr, stop=True)
                    av_ps = av_state[pr]
                    rec = work.tile([HD, 2 * TQ], F32, tag="rec")
                    nc.vector.reciprocal(rec[:], av_ps[HD:2 * HD, :])
                    for sub in range(2):
                        nc.vector.tensor_tensor(
                            aot_sb[pr][sub * HD:(sub + 1) * HD,
                                       qc * TQ:(qc + 1) * TQ],
                            av_ps[0:HD, sub * TQ:(sub + 1) * TQ],
                            rec[:, sub * TQ:(sub + 1) * TQ], op=ALU.mult)

            # ================= Phase 3: output projection =================
            zo_state = {}

            def emit_p3h(lt, h, zeng=None):
                """Half of the output projection for l-tile lt (2 of 4 dc)."""
                if h == 0:
                    zo_state[lt] = zp.tile([128, D], BF16, tag="zo",
                                           name="zo")
                zo = zo_state[lt]
                for dc in (2 * h, 2 * h + 1):
                    z_ps = ps_mm.tile([128, 512], F32, tag="mm", name="z_ps")
                    for et in range(2):
                        nc.tensor.matmul(
                            z_ps[:], aot_sb[et][:, lt * 128:(lt + 1) * 128],
                            wo_sb[et][:, dc * 512:(dc + 1) * 512],
                            start=(et == 0), stop=(et == 1))
                    zslice = zo[:, dc * 512:(dc + 1) * 512]
                    if (zeng or KOPT_ZENG[dc]) == "a":
                        nc.scalar.copy(zslice, z_ps[:])
                    else:
                        nc.vector.tensor_copy(zslice, z_ps[:])
                if h == 1:
                    del zo_state[lt]
                    nc.scalar.dma_start(y[lt * 128:(lt + 1) * 128, :], zo[:])

            def emit_p3(lt):
                emit_p3h(lt, 0)
                emit_p3h(lt, 1)

            def weave(p2l, p1l, p3l, zeng=None):
                """Emit p2 block units as the spine; p1 units are spread over
                the front of the spine (they feed the NEXT stage), p3 units
                over the back (they consume THIS stage's early results)."""
                p3u = [(lt, h) for lt in p3l for h in (0, 1)]
                if not KOPT_WEAVE:
                    for lt in p1l:
                        emit_p1(lt)
                    for u in p2l:
                        emit_p2_unit(u)
                    for lt, h in p3u:
                        emit_p3h(lt, h)
                    return
                others = []
                n1, n3 = len(p1l), len(p3u)
                for i, lt in enumerate(p1l):
                    frac = 0.06 + (i + 0.5) / n1 * 0.62 if n1 else 0.0
                    others.append((frac, "p1", lt))
                for i, u in enumerate(p3u):
                    frac = 0.30 + (i + 0.5) / n3 * 0.66 if n3 else 0.0
                    others.append((frac, "p3", u))
                others.sort()
                # flush pending transposes: the first p2 blocks need them
                emit_transposes()
                n2, no = len(p2l), len(others)
                oi = 0

                def emit_other(o):
                    if o[1] == "p1":
                        emit_p1(o[2])
                    else:
                        emit_p3h(*o[2], zeng=zeng)

                for i, u in enumerate(p2l):
                    emit_p2_unit(u)
                    while oi < no and others[oi][0] <= (i + 1.0) / n2:
                        emit_other(others[oi])
                        oi += 1
                while oi < no:
                    emit_other(others[oi])
                    oi += 1

            def emit_body():
                # Rotated steady-state schedule: stage S3 runs the NEXT
                # iteration's p1(0..3) and S0 the PREVIOUS iteration's
                # p3(12..15); prologue/epilogue peel the ends. p1 units
                # process two l-tiles each.
                weave(p2_units(0), [4, 5, 6, 7], [12, 13, 14, 15])
                weave(p2_units(1), [8, 9, 10, 11], [0, 1, 2, 3])
                weave(p2_units(2), [12, 13, 14, 15], [4, 5, 6, 7])
                weave(p2_units(3), [0, 1, 2, 3], [8, 9, 10, 11])

            # preamble: define aot q3 cols so iter-0's rotated p3 reads zeros
            for et in range(2):
                nc.vector.memset(aot_sb[et][:, 3 * TQ:L], 0.0)
            # prologue: first iteration's p1(0..3)
            for lt in range(4):
                emit_p1(lt)
            if repeat > 1 and os.environ.get("KOPT_UNROLL") == "1":
                for _ in range(repeat):
                    emit_body()
            elif repeat > 1:
                # partial unroll: the For_i back-edge drains every engine
                # (full barrier) once per trip; unrolling U bodies per trip
                # amortizes it
                unroll = int(os.environ.get("KOPT_LOOPUNROLL", "3"))
                while repeat % unroll:
                    unroll -= 1
                with tc.For_i(0, repeat // unroll, 1,
                              staggered_reset=os.environ.get(
                                  "KOPT_STAGRESET", "0") == "1"):
                    for _ in range(unroll):
                        emit_body()
            else:
                emit_body()
            # epilogue: last iteration's p3(12..15)
            for lt in range(12, 16):
                emit_p3(lt)

    nc.compile()
    return nc


_PROGRAM_CACHE = {}


def _get_program(kinds=None, n_mixed=0, repeat=1, deltas=None, W_FOLDED=False):
    key = repeat
    if key not in _PROGRAM_CACHE:
        _PROGRAM_CACHE[key] = _build_program(repeat)
    return _PROGRAM_CACHE[key]


def _host_prep(x, W_qkv, W_out, q_norm_w, k_norm_w, mask):
    # RoPE tables, tiled [128, LT*32]: cos2[p, lt*32+j] = cos((lt*128+p)*freq_j)
    j = np.arange(0, HD, 2, dtype=np.float32)
    freqs = (ROPE_BASE ** (-j / HD)).astype(np.float32)
    pos = np.arange(L, dtype=np.float32)
    theta = pos[:, None] * freqs[None, :]
    cosf = np.cos(theta).astype(np.float32)     # [L, 32]
    sinf = np.sin(theta).astype(np.float32)
    cos2 = np.ascontiguousarray(
        cosf.reshape(LT, 128, 32).transpose(1, 0, 2).reshape(128, LT * 32))
    sin2 = np.ascontiguousarray(
        sinf.reshape(LT, 128, 32).transpose(1, 0, 2).reshape(128, LT * 32))

    scale = np.float32(HD ** -0.5)
    qwv = (np.asarray(q_norm_w, np.float32) * scale)
    kwv = np.asarray(k_norm_w, np.float32)
    qw_rep = np.tile(np.tile(qwv, GQ)[None, :], (128, 1)).astype(np.float32)
    kw_rep = np.tile(kwv[None, :], (128, 1))

    # the kernel hardcodes the causal structure; verify the mask matches
    mref = np.triu(np.full((L, L), NEG, dtype=np.float32), k=1)
    assert np.array_equal(np.asarray(mask, np.float32), mref), \
        "kernel is specialized to the causal mask"
    r = np.arange(128)
    tri = np.where(r[None, :] >= r[:, None], 0.0, NEG).astype(bfloat16)

    in_maps = []
    for c in range(N_CORES):
        b, g = divmod(c, KV)
        xb = np.asarray(x[b], np.float32)
        # xt[lt, p, dt*128 + j] = x[lt*128 + j, dt*128 + p]
        xt = np.ascontiguousarray(
            xb.reshape(LT, 128, DT, 128).transpose(0, 3, 2, 1)
            .reshape(LT, 128, D)).astype(bfloat16)
        rows = np.r_[g * GQ * HD:(g + 1) * GQ * HD,
                     (H + g) * HD:(H + g + 1) * HD,
                     (H + KV + g) * HD:(H + KV + g + 1) * HD]
        wqkT = np.ascontiguousarray(
            np.asarray(W_qkv, np.float32)[rows].T).astype(bfloat16)
        cols = np.arange(g * GQ * HD, (g + 1) * GQ * HD)
        woT = np.ascontiguousarray(
            np.asarray(W_out, np.float32)[:, cols].T).astype(bfloat16)
        m = {"xt": xt, "wqkT": wqkT, "woT": woT,
             "cos2": cos2, "sin2": sin2, "qw": qw_rep, "kw": kw_rep,
             "tri": tri}
        in_maps.append(m)
    return None, 0, in_maps, None, False


def kernel(x, W_qkv, W_out, q_norm_w, k_norm_w, mask):
    kinds, n_mixed, in_maps, deltas, wf = _host_prep(x, W_qkv, W_out,
                                                     q_norm_w, k_norm_w, mask)
    nc = _get_program(repeat=1)
    res = bass_utils.run_bass_kernel_spmd(nc, in_maps,
                                          core_ids=list(range(N_CORES)))
    out = np.zeros((B, L, D), dtype=np.float32)
    for c in range(N_CORES):
        b = c // KV
        out[b] += res.results[c]["y"].astype(np.float32)
    return out


# revision 37
# speedup vs baseline: 1.0632x; 1.0632x over previous
"""Trainium2 Bass kernel for fused GQA attention block (B=2, L=2048, D=2048,
H=16 q-heads, KV=4 kv-heads, HD=64, causal, QK-RMSNorm + RoPE).

Sharding (8 cores): core c -> batch b = c // 4, head-group g = c % 4
(query heads 4g..4g+3, kv head g). Each core computes its 4 heads'
attention and a partial output projection (256 of 1024 e-channels);
host sums the 4 partials per batch (outside the timed device program).

v2 design (vs baseline):
 - bf16 operands everywhere off-PSUM (x, weights, Q^T/K^T/V, p, aot, y);
   fp32 PSUM accumulation. Halves DMA and SBUF traffic; same PE rate.
 - pair transposes: one [128,128] is_transpose matmul per head-pair (and a
   duplicated-k one) directly yields the pair-stacked layout; single DVE
   copy into an interleaved persistent Q^T/K^T buffer.
 - causal masking via a single [128,128] triangular constant accumulated
   on the PE into the boundary sub-tile of diagonal score blocks; exp and
   the score/AV matmuls are trimmed to the valid q-window. No gpsimd
   affine_select, no DVE mask adds.
 - rstd = exp(-0.5*ln(ms+eps)): keeps every activation (Square, Ln, Exp,
   Copy) in one act table set -> no table reloads when phases interleave.
 - phases are software-pipelined at q-chunk granularity: attention of
   chunk qc is interleaved with QKV+rope of chunk qc+1 and the output
   projection of chunk qc-1, so the PE never sits behind softmax exps.
"""

import os

import numpy as np
from ml_dtypes import bfloat16

import concourse.bass as bass
import concourse.mybir as mybir
import concourse.tile as tile
from concourse import bacc
from concourse import bass_utils
from concourse.hw_specs import get_activation_tables
from concourse.masks import make_identity

F32 = mybir.dt.float32
BF16 = mybir.dt.bfloat16
AF = mybir.ActivationFunctionType
ALU = mybir.AluOpType

B, L, D = 2, 2048, 2048
H, KV, HD = 16, 4, 64
EPS = 1e-6
ROPE_BASE = 10000.0
N_CORES = 8
GQ = H // KV          # 4 query heads per core
LT = L // 128         # 16 l-tiles
DT = D // 128         # 16 d-tiles (contraction tiles for qkv proj)
TQ = 512              # q-chunk width for attention
NQC = L // TQ         # 4 q-chunks
NKB = L // 128        # 16 k-blocks
G5 = GQ + 1           # norm groups (4 q heads + 1 k head)
EW = (GQ + 2) * HD    # 384 qkv channels per core
EO = GQ * HD          # 256 output channels per core
NEG = -1e9

KOPT_ZENG = os.environ.get("KOPT_ZENG", "vava")  # staging engine per dc
# (gpsimd cannot read PSUM, so only v=DVE / a=Act are valid here)
KOPT_WEAVE = os.environ.get("KOPT_WEAVE", "1") == "1"


def _build_program(repeat=1):
    nc = bacc.Bacc("TRN2", target_bir_lowering=False, debug=False,
                   enable_asserts=False, num_devices=N_CORES)

    # DRAM I/O (per core). Host pre-tiles everything into DMA-friendly
    # fully-contiguous bf16 layouts.
    xt = nc.dram_tensor("xt", [LT, 128, D], BF16, kind="ExternalInput").ap()
    wqkT = nc.dram_tensor("wqkT", [D, EW], BF16, kind="ExternalInput").ap()
    woT = nc.dram_tensor("woT", [EO, D], BF16, kind="ExternalInput").ap()
    cos2 = nc.dram_tensor("cos2", [128, LT * 32], F32, kind="ExternalInput").ap()
    sin2 = nc.dram_tensor("sin2", [128, LT * 32], F32, kind="ExternalInput").ap()
    qw = nc.dram_tensor("qw", [128, GQ * HD], F32, kind="ExternalInput").ap()
    kw = nc.dram_tensor("kw", [128, HD], F32, kind="ExternalInput").ap()
    tri = nc.dram_tensor("tri", [128, 128], BF16, kind="ExternalInput").ap()
    y = nc.dram_tensor("y", [L, D], BF16, kind="ExternalOutput").ap()

    with tile.TileContext(nc) as tc:
        with (
            tc.tile_pool(name="consts", bufs=1) as consts,
            tc.tile_pool(name="wpool", bufs=1) as wpool,
            tc.tile_pool(name="xcolp", bufs=3) as xcolp,
            tc.tile_pool(name="work", bufs=4) as work,
            tc.tile_pool(name="persist", bufs=1) as persist,
            tc.tile_pool(name="pp", bufs=4) as pp,
            tc.tile_pool(name="zp", bufs=2) as zp,
            tc.tile_pool(name="ps_mm", bufs=2, space="PSUM") as ps_mm,
            tc.tile_pool(name="ps_sc", bufs=2, space="PSUM") as ps_sc,
            tc.tile_pool(name="ps_av", bufs=1, space="PSUM") as ps_av,
        ):
            # Pin the one act table that serves Square+Ln+Exp+Copy; the
            # auto-inserter would otherwise thrash between per-func tables
            # (1283ns per reload) in the interleaved schedule.
            tabs = list(get_activation_tables(nc.m.arch).items())
            set_id = next(i for i, (_n, fns) in enumerate(tabs)
                          if {AF.Ln, AF.Exp, AF.Square, AF.Copy} <= fns)
            nc.scalar.add_instruction(mybir.InstLoadActFuncSet(
                name=f"I-{nc.next_id()}", engine=mybir.EngineType.Activation,
                act_func_set_id=set_id))

            # ---- constants ----
            ident_b = consts.tile([128, 128], BF16, tag="identb")
            make_identity(nc, ident_b[:])
            tri_sb = consts.tile([128, 128], BF16, tag="tri")
            nc.sync.dma_start(tri_sb[:], tri[:])
            cos_sb = consts.tile([128, LT * 32], F32, tag="cos")
            sin_sb = consts.tile([128, LT * 32], F32, tag="sin")
            nc.sync.dma_start(cos_sb[:], cos2[:])
            nc.sync.dma_start(sin_sb[:], sin2[:])
            eps_sb = consts.tile([128, 1], F32, tag="eps")
            nc.vector.memset(eps_sb[:], EPS)
            # w5 = [qw x4 | kw] (qw already has HD^-0.5 folded in on host)
            w5_sb = consts.tile([128, G5 * HD], F32, tag="w5")
            nc.sync.dma_start(w5_sb[:, 0:GQ * HD], qw[:])
            nc.sync.dma_start(w5_sb[:, GQ * HD:G5 * HD], kw[:])
            # bf16 copies of the rope/norm constants: all-2-byte operands
            # unlock the DVE 2x perf mode on the rope chain
            w5_b = consts.tile([128, G5 * HD], BF16, tag="w5b")
            nc.vector.tensor_copy(w5_b[:], w5_sb[:])
            cos_b = consts.tile([128, LT * 32], BF16, tag="cosb")
            sin_b = consts.tile([128, LT * 32], BF16, tag="sinb")
            nc.vector.tensor_copy(cos_b[:], cos_sb[:])
            nc.vector.tensor_copy(sin_b[:], sin_sb[:])

            # ---- weights (bf16) ----
            wqk_sb = []
            for dt_i in range(DT):
                w = wpool.tile([128, EW], BF16, tag=f"wqk{dt_i}")
                nc.sync.dma_start(w[:], wqkT[dt_i * 128:(dt_i + 1) * 128, :])
                wqk_sb.append(w)
            wo_sb = []
            for et in range(2):
                w = wpool.tile([128, D], BF16, tag=f"wo{et}")
                nc.sync.dma_start(w[:], woT[et * 128:(et + 1) * 128, :])
                wo_sb.append(w)

            # ---- persistent attention operands ----
            # qk_all[p, lt, g, j]: g=0 -> pair0 (heads 0,1 chan-stacked),
            # g=1 -> pair1 (heads 2,3), g=2 -> k^T duplicated on both halves.
            qk_all = persist.tile([128, LT * 3 * 128], BF16, tag="qkall")
            qk_v = qk_all[:].rearrange("p (t g j) -> p t g j", g=3, j=128)
            # vt[p=k-pos, (lt, hd|ones)]
            vt_sb = persist.tile([128, LT * 128], BF16, tag="vt")
            ones_sb = consts.tile([128, HD], BF16, tag="ones")
            nc.vector.memset(ones_sb[:], 1.0)
            for i in range(LT):
                nc.vector.tensor_copy(
                    vt_sb[:, i * 128 + HD:(i + 1) * 128], ones_sb[:])
            aot_sb = [persist.tile([128, L], BF16, tag=f"aot{et}",
                                   name=f"aot{et}")
                      for et in range(2)]

            state = {"prev": None}

            def emit_transposes():
                if state["prev"] is None:
                    return
                lt, rq = state["prev"]
                state["prev"] = None
                tp = ps_mm.tile([128, 1024], BF16, tag="mm", name="tp")
                for g in range(3):
                    nc.tensor.matmul(
                        tp[:, g * 128:(g + 1) * 128],
                        rq[:, g * 128:(g + 1) * 128],
                        ident_b[:], is_transpose=True,
                        skip_group_check=True)
                nc.vector.tensor_copy(
                    qk_v[:, lt, :, :],
                    tp[:, 0:384].rearrange("p (g j) -> p g j", j=128))

            # ================= Phase 1: QKV + RMSNorm + RoPE =================
            def emit_p1(lt):
                xcol = xcolp.tile([128, D], BF16, tag="xcol")
                nc.sync.dma_start(xcol[:], xt[lt, :, :])
                qkv_ps = ps_mm.tile([128, 512], F32, tag="mm", name="qkv_ps")
                for dt_i in range(DT):
                    nc.tensor.matmul(
                        qkv_ps[:, 0:EW],
                        xcol[:, dt_i * 128:(dt_i + 1) * 128],
                        wqk_sb[dt_i][:],
                        start=(dt_i == 0), stop=(dt_i == DT - 1))
                emit_transposes()

                # stage q,k out of PSUM; V straight to persistent (bf16)
                q5 = work.tile([128, G5 * HD], F32, tag="q5")
                nc.vector.tensor_copy(q5[:], qkv_ps[:, 0:G5 * HD])
                nc.vector.tensor_copy(
                    vt_sb[:, lt * 128:lt * 128 + HD],
                    qkv_ps[:, G5 * HD:(G5 + 1) * HD])

                # RMS stats: one act Square + one DVE X-reduce
                sq = work.tile([128, G5 * HD], F32, tag="sq")
                nc.scalar.activation(sq[:], q5[:], AF.Square)
                ss = work.tile([128, 16], F32, tag="ss")
                nc.vector.tensor_reduce(
                    ss[:, 0:G5],
                    sq[:].rearrange("p (h e) -> p h e", e=HD),
                    axis=mybir.AxisListType.X, op=ALU.add)
                # rstd = exp(-0.5 * ln(ss/HD + eps)); Ln+Exp share the act
                # table with Square/Copy -> no table reloads anywhere.
                nc.scalar.activation(ss[:, 5:5 + G5], ss[:, 0:G5],
                                     AF.Ln, bias=eps_sb[:], scale=1.0 / HD)
                nc.scalar.activation(ss[:, 10:10 + G5], ss[:, 5:5 + G5],
                                     AF.Exp, scale=-0.5)

                # normalize * weight (5 groups at once); bf16 from here on
                # so the rope tensor_tensor chain runs in DVE 2x mode
                qn = work.tile([128, G5 * HD], BF16, tag="qn")
                nc.vector.tensor_tensor(
                    qn[:].rearrange("p (h e) -> p h e", e=HD),
                    q5[:].rearrange("p (h e) -> p h e", e=HD),
                    ss[:, 10:10 + G5, None].broadcast_to([128, G5, HD]),
                    op=ALU.mult)
                nc.vector.tensor_tensor(qn[:], qn[:], w5_b[:], op=ALU.mult)

                # RoPE on all 5 groups; outputs bf16 for 1cyc/row transposes
                cs = cos_b[:, lt * 32:(lt + 1) * 32]
                sn = sin_b[:, lt * 32:(lt + 1) * 32]
                csq = cs[:, None, :].broadcast_to([128, G5, 32])
                snq = sn[:, None, :].broadcast_to([128, G5, 32])
                rq = work.tile([128, 3 * 128], BF16, tag="rq")
                rqv = rq[:, 0:G5 * HD].rearrange("p (h e) -> p h e", e=HD)
                qnv = qn[:].rearrange("p (h e) -> p h e", e=HD)
                t1 = work.tile([128, G5 * 32], BF16, tag="t1")
                t1v = t1[:].rearrange("p (h e) -> p h e", e=32)
                t2 = work.tile([128, G5 * 32], BF16, tag="t2")
                t2v = t2[:].rearrange("p (h e) -> p h e", e=32)
                # low half: x1*cos - x2*sin
                nc.vector.tensor_tensor(t1v, qnv[:, :, 0:32], csq, op=ALU.mult)
                nc.vector.tensor_tensor(t2v, qnv[:, :, 32:64], snq, op=ALU.mult)
                nc.vector.tensor_tensor(rqv[:, :, 0:32], t1v, t2v,
                                        op=ALU.subtract)
                # high half: x1*sin + x2*cos
                nc.vector.tensor_tensor(t1v, qnv[:, :, 0:32], snq, op=ALU.mult)
                nc.vector.tensor_tensor(t2v, qnv[:, :, 32:64], csq, op=ALU.mult)
                nc.vector.tensor_tensor(rqv[:, :, 32:64], t1v, t2v, op=ALU.add)
                # duplicate roped k so its transpose fills both halves
                nc.vector.tensor_copy(rq[:, 320:384], rq[:, 256:320])

                state["prev"] = (lt, rq)

            # ================= Phase 2: attention =================
            # Spine is software-pipelined one block deep: the AV matmul for
            # block ci is emitted after the scores+exp of block ci+1, so the
            # PE streams the next block's scores while the act engine exps
            # the current one.
            av_state = {}
            pend_state = {}

            def p2_units(qc):
                units = []
                for pr in range(GQ // 2):
                    nkb = 4 * qc + 4  # causal: kb in [0, 4qc+3]
                    for ci in range(nkb):
                        units.append(("blk", pr, qc, ci, nkb))
                    units.append(("norm", pr, qc))
                return units

            def emit_av(pr, stop):
                ci, nkb, p_sb, kb, delta = pend_state.pop(pr)
                av_ps = av_state[pr]
                for sub in range(2):
                    nc.tensor.matmul(
                        av_ps[:, sub * TQ + delta:(sub + 1) * TQ],
                        vt_sb[:, kb * 128:(kb + 1) * 128],
                        p_sb[:, sub * TQ + delta:(sub + 1) * TQ],
                        start=(ci == 0), stop=stop)

            def emit_p2_unit(u):
                kind = u[0]
                if kind == "blk":
                    _, pr, qc, ci, nkb = u
                    kb = ci
                    delta = max(0, kb * 128 - qc * TQ)
                    diag = kb >= 4 * qc
                    if ci == 0:
                        av_state[pr] = ps_av.tile([128, 2 * TQ], F32, tag="av",
                                                  name=f"av{pr}")
                    sc_ps = ps_sc.tile([128, 2 * TQ], F32, tag="sc")
                    for sub in range(2):
                        qsl = qk_v[sub * 64:(sub + 1) * 64,
                                   4 * qc + delta // 128:4 * qc + 4, pr, :]
                        nc.tensor.matmul(
                            sc_ps[:, sub * TQ + delta:(sub + 1) * TQ],
                            qk_v[sub * 64:(sub + 1) * 64, kb, 2, :],
                            qsl,
                            start=True, stop=not diag)
                        if diag:
                            nc.tensor.matmul(
                                sc_ps[:, sub * TQ + delta:sub * TQ + delta + 128],
                                ident_b[:], tri_sb[:],
                                start=False, stop=True)
                    p_sb = pp.tile([128, 2 * TQ], BF16, tag="p")
                    sc3 = sc_ps[:].rearrange("p (s q) -> p s q", q=TQ)
                    p3 = p_sb[:].rearrange("p (s q) -> p s q", q=TQ)
                    nc.scalar.activation(p3[:, :, delta:TQ],
                                         sc3[:, :, delta:TQ], AF.Exp)
                    if pr in pend_state:
                        emit_av(pr, stop=False)
                    pend_state[pr] = (ci, nkb, p_sb, kb, delta)
                else:
                    _, pr, qc = u
                    emit_av(pr, stop=True)
                    av_ps = av_state[pr]
                    rec = work.tile([HD, 2 * TQ], F32, tag="rec")
                    nc.vector.reciprocal(rec[:], av_ps[HD:2 * HD, :])
                    for sub in range(2):
                        nc.vector.tensor_tensor(
                            aot_sb[pr][sub * HD:(sub + 1) * HD,
                                       qc * TQ:(qc + 1) * TQ],
                            av_ps[0:HD, sub * TQ:(sub + 1) * TQ],
                            rec[:, sub * TQ:(sub + 1) * TQ], op=ALU.mult)

            # ================= Phase 3: output projection =================
            zo_state = {}

            def emit_p3h(lt, h, zeng=None):
                """Half of the output projection for l-tile lt (2 of 4 dc)."""
                if h == 0:
                    zo_state[lt] = zp.tile([128, D], BF16, tag="zo",
                                           name="zo")
                zo = zo_state[lt]
                for dc in (2 * h, 2 * h + 1):
                    z_ps = ps_mm.tile([128, 512], F32, tag="mm", name="z_ps")
                    for et in range(2):
                        nc.tensor.matmul(
                            z_ps[:], aot_sb[et][:, lt * 128:(lt + 1) * 128],
                            wo_sb[et][:, dc * 512:(dc + 1) * 512],
                            start=(et == 0), stop=(et == 1))
                    zslice = zo[:, dc * 512:(dc + 1) * 512]
                    if (zeng or KOPT_ZENG[dc]) == "a":
                        nc.scalar.copy(zslice, z_ps[:])
                    else:
                        nc.vector.tensor_copy(zslice, z_ps[:])
                if h == 1:
                    del zo_state[lt]
                    nc.scalar.dma_start(y[lt * 128:(lt + 1) * 128, :], zo[:])

            def emit_p3(lt):
                emit_p3h(lt, 0)
                emit_p3h(lt, 1)

            def weave(p2l, p1l, p3l, zeng=None):
                """Emit p2 block units as the spine; p1 units are spread over
                the front of the spine (they feed the NEXT stage), p3 units
                over the back (they consume THIS stage's early results)."""
                p3u = [(lt, h) for lt in p3l for h in (0, 1)]
                if not KOPT_WEAVE:
                    for lt in p1l:
                        emit_p1(lt)
                    for u in p2l:
                        emit_p2_unit(u)
                    for lt, h in p3u:
                        emit_p3h(lt, h)
                    return
                others = []
                n1, n3 = len(p1l), len(p3u)
                f1a = float(os.environ.get("KOPT_F1A", "0.06"))
                f1b = float(os.environ.get("KOPT_F1B", "0.62"))
                f3a = float(os.environ.get("KOPT_F3A", "0.30"))
                f3b = float(os.environ.get("KOPT_F3B", "0.66"))
                for i, lt in enumerate(p1l):
                    frac = f1a + (i + 0.5) / n1 * f1b if n1 else 0.0
                    others.append((frac, "p1", lt))
                for i, u in enumerate(p3u):
                    frac = f3a + (i + 0.5) / n3 * f3b if n3 else 0.0
                    others.append((frac, "p3", u))
                others.sort()
                # flush pending transposes: the first p2 blocks need them
                emit_transposes()
                n2, no = len(p2l), len(others)
                oi = 0

                def emit_other(o):
                    if o[1] == "p1":
                        emit_p1(o[2])
                    else:
                        emit_p3h(*o[2], zeng=zeng)

                for i, u in enumerate(p2l):
                    emit_p2_unit(u)
                    while oi < no and others[oi][0] <= (i + 1.0) / n2:
                        emit_other(others[oi])
                        oi += 1
                while oi < no:
                    emit_other(others[oi])
                    oi += 1

            def emit_body():
                # Rotated steady-state schedule: stage S3 runs the NEXT
                # iteration's p1(0..3) and S0 the PREVIOUS iteration's
                # p3(12..15); prologue/epilogue peel the ends. p1 units
                # process two l-tiles each.
                weave(p2_units(0), [4, 5, 6, 7], [12, 13, 14, 15])
                weave(p2_units(1), [8, 9, 10, 11], [0, 1, 2, 3])
                weave(p2_units(2), [12, 13, 14, 15], [4, 5, 6, 7])
                weave(p2_units(3), [0, 1, 2, 3], [8, 9, 10, 11])

            # preamble: define aot q3 cols so iter-0's rotated p3 reads zeros
            for et in range(2):
                nc.vector.memset(aot_sb[et][:, 3 * TQ:L], 0.0)
            # prologue: first iteration's p1(0..3)
            for lt in range(4):
                emit_p1(lt)
            if repeat > 1 and os.environ.get("KOPT_UNROLL") == "1":
                for _ in range(repeat):
                    emit_body()
            elif repeat > 1:
                # partial unroll: the For_i back-edge drains every engine
                # (full barrier) once per trip; unrolling U bodies per trip
                # amortizes it
                unroll = int(os.environ.get("KOPT_LOOPUNROLL", "3"))
                while repeat % unroll:
                    unroll -= 1
                with tc.For_i(0, repeat // unroll, 1,
                              staggered_reset=os.environ.get(
                                  "KOPT_STAGRESET", "0") == "1"):
                    for _ in range(unroll):
                        emit_body()
            else:
                emit_body()
            # epilogue: last iteration's p3(12..15)
            for lt in range(12, 16):
                emit_p3(lt)

    nc.compile()
    return nc


_PROGRAM_CACHE = {}


def _get_program(kinds=None, n_mixed=0, repeat=1, deltas=None, W_FOLDED=False):
    key = repeat
    if key not in _PROGRAM_CACHE:
        _PROGRAM_CACHE[key] = _build_program(repeat)
    return _PROGRAM_CACHE[key]


def _host_prep(x, W_qkv, W_out, q_norm_w, k_norm_w, mask):
    # RoPE tables, tiled [128, LT*32]: cos2[p, lt*32+j] = cos((lt*128+p)*freq_j)
    j = np.arange(0, HD, 2, dtype=np.float32)
    freqs = (ROPE_BASE ** (-j / HD)).astype(np.float32)
    pos = np.arange(L, dtype=np.float32)
    theta = pos[:, None] * freqs[None, :]
    cosf = np.cos(theta).astype(np.float32)     # [L, 32]
    sinf = np.sin(theta).astype(np.float32)
    cos2 = np.ascontiguousarray(
        cosf.reshape(LT, 128, 32).transpose(1, 0, 2).reshape(128, LT * 32))
    sin2 = np.ascontiguousarray(
        sinf.reshape(LT, 128, 32).transpose(1, 0, 2).reshape(128, LT * 32))

    scale = np.float32(HD ** -0.5)
    qwv = (np.asarray(q_norm_w, np.float32) * scale)
    kwv = np.asarray(k_norm_w, np.float32)
    qw_rep = np.tile(np.tile(qwv, GQ)[None, :], (128, 1)).astype(np.float32)
    kw_rep = np.tile(kwv[None, :], (128, 1))

    # the kernel hardcodes the causal structure; verify the mask matches
    mref = np.triu(np.full((L, L), NEG, dtype=np.float32), k=1)
    assert np.array_equal(np.asarray(mask, np.float32), mref), \
        "kernel is specialized to the causal mask"
    r = np.arange(128)
    tri = np.where(r[None, :] >= r[:, None], 0.0, NEG).astype(bfloat16)

    in_maps = []
    for c in range(N_CORES):
        b, g = divmod(c, KV)
        xb = np.asarray(x[b], np.float32)
        # xt[lt, p, dt*128 + j] = x[lt*128 + j, dt*128 + p]
        xt = np.ascontiguousarray(
            xb.reshape(LT, 128, DT, 128).transpose(0, 3, 2, 1)
            .reshape(LT, 128, D)).astype(bfloat16)
        rows = np.r_[g * GQ * HD:(g + 1) * GQ * HD,
                     (H + g) * HD:(H + g + 1) * HD,
                     (H + KV + g) * HD:(H + KV + g + 1) * HD]
        wqkT = np.ascontiguousarray(
            np.asarray(W_qkv, np.float32)[rows].T).astype(bfloat16)
        cols = np.arange(g * GQ * HD, (g + 1) * GQ * HD)
        woT = np.ascontiguousarray(
            np.asarray(W_out, np.float32)[:, cols].T).astype(bfloat16)
        m = {"xt": xt, "wqkT": wqkT, "woT": woT,
             "cos2": cos2, "sin2": sin2, "qw": qw_rep, "kw": kw_rep,
             "tri": tri}
        in_maps.append(m)
    return None, 0, in_maps, None, False


def kernel(x, W_qkv, W_out, q_norm_w, k_norm_w, mask):
    kinds, n_mixed, in_maps, deltas, wf = _host_prep(x, W_qkv, W_out,
                                                     q_norm_w, k_norm_w, mask)
    nc = _get_program(repeat=1)
    res = bass_utils.run_bass_kernel_spmd(nc, in_maps,
                                          core_ids=list(range(N_CORES)))
    out = np.zeros((B, L, D), dtype=np.float32)
    for c in range(N_CORES):
        b = c // KV
        out[b] += res.results[c]["y"].astype(np.float32)
    return out
